# revision 7
# baseline (speedup 1.0000x reference)
"""MoE FFN (E=8 experts, top-2) — expert-parallel Bass/Tile kernel for 8 TRN2 cores.

Strategy:
  - Host computes the (tiny) router: logits = x @ gate_w.T, top-2 per token,
    renormalized weights (= sigmoid of logit differences).  This is the
    sharding decision: token n is dispatched to cores e1(n), e2(n).
  - Core e receives its expert's weights (pre-tiled) and the gathered,
    transposed tokens XgT [D, C] plus per-token gate weights.
  - Device: hT = gelu(w1.T @ xgT + b1)  (feature-major), then
    Y = hT.T @ w2 scaled by gate (fused in PSUM eviction).  Matmuls run as
    float32r (full-rate fp32, ~11-bit mantissa; PE rounds operands itself).
  - Host combine: out[idx_e] += Yg_e (each token appears in exactly 2 experts,
    never twice in one), plus the gate-weighted b2 term.
"""

import numpy as np

import concourse.bass as bass
import concourse.mybir as mybir
import concourse.tile as tile
from concourse import bacc, bass_utils

P = 128
D_MODEL = 1024
D_HID = 4096
E = 8
TOP_K = 2
N_CORES = 8

DC = D_MODEL // P          # 8 d-chunks (contraction for mm1)
HC = D_HID // P            # 32 h-chunks
HG = 4                     # h-chunks per w1 load group
NHG = HC // HG             # 8 groups
C = 1280                   # per-expert token capacity (>= max expert load)
NB = 2                     # token blocks
BT = C // NB               # 640 tokens per block
NSUB = 2                   # mm1 moving-dim subtiles per block
SUB = BT // NSUB           # 320 (>=256 keeps float32r at full rate)
NCH = BT // P              # 5 token chunks per block for mm2
DT = 2                     # output d tiles
DTS = D_MODEL // DT        # 512

F32 = mybir.dt.float32
F32R = mybir.dt.float32r
MM_DT = F32R               # matmul operand dtype (F32R = fast, F32 = exact)


def build_nc():
    nc = bacc.Bacc("TRN2", target_bir_lowering=False, debug=False,
                   num_devices=N_CORES)

    # Inputs, pre-tiled on host into consumption order (all contiguous DMAs):
    #   xgt  [DC, P, C]         xgt[dc, p, n] = Xg[n, dc*128+p]
    #   w1t  [NHG, DC, P, HG*P] w1t[hg, dc, p, k*128+j] = w1[dc*128+p, (hg*4+k)*128+j]
    #   w2t  [DT, HC, P, DTS]   w2t[dt, hc, p, j] = w2[hc*128+p, dt*512+j]
    #   b1t  [P, HC]            b1t[p, hc] = b1[hc*128+p]
    #   gt   [P, C//P]          gt[p, c] = gate[c*128+p]
    xgt = nc.dram_tensor("xgt", [DC, P, C], MM_DT, kind="ExternalInput")
    w1t = nc.dram_tensor("w1t", [NHG, DC, P, HG * P], MM_DT, kind="ExternalInput")
    w2t = nc.dram_tensor("w2t", [DT, HC, P, DTS], MM_DT, kind="ExternalInput")
    b1t = nc.dram_tensor("b1t", [P, HC], F32, kind="ExternalInput")
    gt = nc.dram_tensor("gt", [P, C // P], F32, kind="ExternalInput")
    yg = nc.dram_tensor("yg", [C, D_MODEL], F32, kind="ExternalOutput")

    with tile.TileContext(nc) as tc:
        with (
            tc.tile_pool(name="const", bufs=1) as const,
            tc.tile_pool(name="xg", bufs=1) as xg_pool,
            tc.tile_pool(name="w1", bufs=16) as w1_pool,
            tc.tile_pool(name="w2", bufs=6) as w2_pool,
            tc.tile_pool(name="ht", bufs=HC + 4) as ht_pool,
            tc.tile_pool(name="yo", bufs=4) as yo_pool,
            tc.tile_pool(name="ps1", bufs=3, space="PSUM") as ps1,
            tc.tile_pool(name="ps2", bufs=NCH, space="PSUM") as ps2,
        ):
            b1_sb = const.tile([P, HC], F32, name="b1sb")
            nc.sync.dma_start(out=b1_sb[:], in_=b1t[:, :])
            g_sb = const.tile([P, C // P], F32, name="gsb")
            nc.sync.dma_start(out=g_sb[:], in_=gt[:, :])

            xg_sb = []
            for dc in range(DC):
                t = xg_pool.tile([P, C], MM_DT, name=f"xg{dc}")
                nc.sync.dma_start(out=t[:], in_=xgt[dc, :, :])
                xg_sb.append(t)

            for b in range(NB):
                n0 = b * BT
                # ---- mm1: hT[hc] = gelu(w1.T @ xgT + b1) ----
                ht_tiles = []
                w1_cache = {}
                for hc in range(HC):
                    hg, k = divmod(hc, HG)
                    ht = ht_pool.tile([P, BT], MM_DT, name="ht")
                    for s in range(NSUB):
                        ps = ps1.tile([P, SUB], F32, name="ps1")
                        for dc in range(DC):
                            if (hg, dc) not in w1_cache:
                                w1_sb = w1_pool.tile([P, HG * P], MM_DT,
                                                     name="w1sb")
                                nc.sync.dma_start(out=w1_sb[:],
                                                  in_=w1t[hg, dc, :, :])
                                w1_cache[(hg, dc)] = w1_sb
                            w1_sb = w1_cache[(hg, dc)]
                            nc.tensor.matmul(
                                ps[:],
                                lhsT=w1_sb[:, k * P:(k + 1) * P],
                                rhs=xg_sb[dc][:, n0 + s * SUB:n0 + (s + 1) * SUB],
                                start=(dc == 0),
                                stop=(dc == DC - 1),
                            )
                        nc.scalar.activation(
                            ht[:, s * SUB:(s + 1) * SUB], ps[:],
                            mybir.ActivationFunctionType.Gelu,
                            bias=b1_sb[:, hc:hc + 1],
                        )
                    ht_tiles.append(ht)

                # ---- mm2: Y[n0:n0+BT] = (hT.T @ w2) * gate ----
                for dt in range(DT):
                    pss = [ps2.tile([P, DTS], F32, name="ps2")
                           for _ in range(NCH)]
                    for hc in range(HC):
                        w2_sb = w2_pool.tile([P, DTS], MM_DT, name="w2sb")
                        nc.sync.dma_start(out=w2_sb[:], in_=w2t[dt, hc, :, :])
                        for ncq in range(NCH):
                            nc.tensor.matmul(
                                pss[ncq][:],
                                lhsT=ht_tiles[hc][:, ncq * P:(ncq + 1) * P],
                                rhs=w2_sb[:],
                                start=(hc == 0),
                                stop=(hc == HC - 1),
                            )
                    for ncq in range(NCH):
                        yo = yo_pool.tile([P, DTS], F32, name="yo")
                        gcol = (n0 + ncq * P) // P
                        nc.scalar.activation(
                            yo[:], pss[ncq][:],
                            mybir.ActivationFunctionType.Copy,
                            scale=g_sb[:, gcol:gcol + 1],
                        )
                        nc.sync.dma_start(
                            out=yg[n0 + ncq * P:n0 + (ncq + 1) * P,
                                   dt * DTS:(dt + 1) * DTS],
                            in_=yo[:],
                        )
    nc.compile()
    return nc


_NC_CACHE = None
TRACE = False
LAST_RESULTS = None


def _get_nc():
    global _NC_CACHE
    if _NC_CACHE is None:
        _NC_CACHE = build_nc()
    return _NC_CACHE


def kernel(x, gate_w, w1, b1, w2, b2):
    x = np.asarray(x, dtype=np.float32)
    gate_w = np.asarray(gate_w, dtype=np.float32)
    w1 = np.asarray(w1, dtype=np.float32)
    b1 = np.asarray(b1, dtype=np.float32)
    w2 = np.asarray(w2, dtype=np.float32)
    b2 = np.asarray(b2, dtype=np.float32)

    B, T, D = x.shape
    N = B * T
    xf = x.reshape(N, D)

    # ---- router (host; 0.05% of model FLOPs — this is the sharding step) ----
    logits = xf @ gate_w.T                           # [N, E]
    order = np.argsort(-logits, axis=1, kind="stable")
    i1, i2 = order[:, 0], order[:, 1]
    l1 = logits[np.arange(N), i1].astype(np.float64)
    l2 = logits[np.arange(N), i2].astype(np.float64)
    g1 = (1.0 / (1.0 + np.exp(l2 - l1))).astype(np.float32)
    g2 = (1.0 - g1).astype(np.float32)

    # ---- dispatch: gather per-expert tokens, pre-tile all inputs ----
    in_maps = []
    idx_per_e = []
    for e in range(E):
        sel1 = np.nonzero(i1 == e)[0]
        sel2 = np.nonzero(i2 == e)[0]
        idx = np.concatenate([sel1, sel2])
        gv = np.concatenate([g1[sel1], g2[sel2]])
        cnt = idx.shape[0]
        assert cnt <= C, f"expert {e} over capacity: {cnt} > {C}"
        idx_per_e.append(idx)

        xg = np.zeros((C, D), np.float32)
        xg[:cnt] = xf[idx]
        xgt = np.ascontiguousarray(
            xg.T.reshape(DC, P, C))               # [dc, p, n]
        w1t = np.ascontiguousarray(
            w1[e].reshape(DC, P, NHG, HG * P).transpose(2, 0, 1, 3))
        w2t = np.ascontiguousarray(
            w2[e].reshape(HC, P, DT, DTS).transpose(2, 0, 1, 3))
        b1t = np.ascontiguousarray(b1[e].reshape(HC, P).T)
        gfull = np.zeros(C, np.float32)
        gfull[:cnt] = gv
        gt = np.ascontiguousarray(gfull.reshape(C // P, P).T)
        in_maps.append(
            {"xgt": xgt, "w1t": w1t, "w2t": w2t, "b1t": b1t, "gt": gt})

    nc = _get_nc()
    res = bass_utils.run_bass_kernel_spmd(
        nc, in_maps, core_ids=list(range(N_CORES)), trace=TRACE)
    global LAST_RESULTS
    LAST_RESULTS = res

    # ---- combine (host): each token occurs in exactly 2 experts, never twice
    # in one, so fancy-index += is safe per expert ----
    out = np.zeros((N, D), np.float32)
    for e in range(E):
        idx = idx_per_e[e]
        out[idx] += res.results[e]["yg"][:idx.shape[0]]

    if np.any(b2):
        gate_full = np.zeros((N, E), np.float32)
        gate_full[np.arange(N), i1] = g1
        gate_full[np.arange(N), i2] = g2
        out += gate_full @ b2.reshape(E, D)

    return out.reshape(B, T, D)


# revision 11
# speedup vs baseline: 1.0861x; 1.0861x over previous
"""MoE FFN (E=8 experts, top-2) — expert-parallel Bass/Tile kernel for 8 TRN2 cores.

Strategy:
  - Host computes the (tiny) router: logits = x @ gate_w.T, top-2 per token,
    renormalized weights (= sigmoid of logit differences).  This is the
    sharding decision: token n is dispatched to cores e1(n), e2(n).
  - Core e receives its expert's weights (pre-tiled) and the gathered,
    transposed tokens XgT [D, C] plus per-token gate weights.
  - Device: hT = gelu(w1.T @ xgT + b1)  (feature-major), then
    Y = hT.T @ w2 scaled by gate (fused in PSUM eviction).  Matmuls run as
    float32r (full-rate fp32, ~11-bit mantissa; PE rounds operands itself).
  - Host combine: out[idx_e] += Yg_e (each token appears in exactly 2 experts,
    never twice in one), plus the gate-weighted b2 term.
"""

import numpy as np

import concourse.bass as bass
import concourse.mybir as mybir
import concourse.tile as tile
from concourse import bacc, bass_utils

P = 128
D_MODEL = 1024
D_HID = 4096
E = 8
TOP_K = 2
N_CORES = 8

DC = D_MODEL // P          # 8 d-chunks (contraction for mm1)
HC = D_HID // P            # 32 h-chunks
HG = 4                     # h-chunks per w1 load group
NHG = HC // HG             # 8 groups
C = 1152                   # per-expert token capacity (>= max expert load)
# chunk-aligned token blocks (start, size); mm1 subtile sizes stay >=256 so
# float32r runs at full rate, and <=512 for one fp32 PSUM bank
BLOCKS = [(0, 640, (320, 320)), (640, 512, (512,))]
DT = 2                     # output d tiles
DTS = D_MODEL // DT        # 512
MAX_BT = max(bt for _, bt, _ in BLOCKS)

F32 = mybir.dt.float32
F32R = mybir.dt.float32r
MM_DT = F32R               # matmul operand dtype (F32R = fast, F32 = exact)


def build_nc():
    nc = bacc.Bacc("TRN2", target_bir_lowering=False, debug=False,
                   num_devices=N_CORES)

    # Inputs, pre-tiled on host into consumption order (all contiguous DMAs):
    #   xgt  [DC, P, C]         xgt[dc, p, n] = Xg[n, dc*128+p]
    #   w1t  [NHG, DC, P, HG*P] w1t[hg, dc, p, k*128+j] = w1[dc*128+p, (hg*4+k)*128+j]
    #   w2t  [DT, HC, P, DTS]   w2t[dt, hc, p, j] = w2[hc*128+p, dt*512+j]
    #   b1t  [P, HC]            b1t[p, hc] = b1[hc*128+p]
    #   gt   [P, C//P]          gt[p, c] = gate[c*128+p]
    xgt = nc.dram_tensor("xgt", [DC, P, C], MM_DT, kind="ExternalInput")
    w1t = nc.dram_tensor("w1t", [NHG, DC, P, HG * P], MM_DT, kind="ExternalInput")
    w2t = nc.dram_tensor("w2t", [DT, HC, P, DTS], MM_DT, kind="ExternalInput")
    b1t = nc.dram_tensor("b1t", [P, HC], F32, kind="ExternalInput")
    gt = nc.dram_tensor("gt", [P, C // P], F32, kind="ExternalInput")
    yg = nc.dram_tensor("yg", [C, D_MODEL], F32, kind="ExternalOutput")

    with tile.TileContext(nc) as tc:
        with (
            tc.tile_pool(name="const", bufs=1) as const,
            tc.tile_pool(name="xg", bufs=1) as xg_pool,
            tc.tile_pool(name="w1", bufs=16) as w1_pool,
            tc.tile_pool(name="w2", bufs=6) as w2_pool,
            tc.tile_pool(name="ht", bufs=HC + 4) as ht_pool,
            tc.tile_pool(name="yo", bufs=4) as yo_pool,
            tc.tile_pool(name="ps1", bufs=3, space="PSUM") as ps1,
            tc.tile_pool(name="ps2", bufs=MAX_BT // P, space="PSUM") as ps2,
        ):
            b1_sb = const.tile([P, HC], F32, name="b1sb")
            nc.sync.dma_start(out=b1_sb[:], in_=b1t[:, :])
            g_sb = const.tile([P, C // P], F32, name="gsb")
            nc.sync.dma_start(out=g_sb[:], in_=gt[:, :])

            xg_sb = []
            for dc in range(DC):
                t = xg_pool.tile([P, C], MM_DT, name=f"xg{dc}")
                nc.sync.dma_start(out=t[:], in_=xgt[dc, :, :])
                xg_sb.append(t)

            for n0, BT, SUBS in BLOCKS:
                NCH = BT // P
                # ---- mm1: hT[hc] = gelu(w1.T @ xgT + b1) ----
                ht_tiles = []
                w1_cache = {}
                for hc in range(HC):
                    hg, k = divmod(hc, HG)
                    ht = ht_pool.tile([P, MAX_BT], MM_DT, name="ht")
                    sub0 = 0
                    for SUB in SUBS:
                        ps = ps1.tile([P, SUB], F32, name="ps1")
                        for dc in range(DC):
                            if (hg, dc) not in w1_cache:
                                w1_sb = w1_pool.tile([P, HG * P], MM_DT,
                                                     name="w1sb")
                                nc.sync.dma_start(out=w1_sb[:],
                                                  in_=w1t[hg, dc, :, :])
                                w1_cache[(hg, dc)] = w1_sb
                            w1_sb = w1_cache[(hg, dc)]
                            nc.tensor.matmul(
                                ps[:],
                                lhsT=w1_sb[:, k * P:(k + 1) * P],
                                rhs=xg_sb[dc][:, n0 + sub0:n0 + sub0 + SUB],
                                start=(dc == 0),
                                stop=(dc == DC - 1),
                            )
                        nc.scalar.activation(
                            ht[:, sub0:sub0 + SUB], ps[:],
                            mybir.ActivationFunctionType.Gelu,
                            bias=b1_sb[:, hc:hc + 1],
                        )
                        sub0 += SUB
                    ht_tiles.append(ht)

                # ---- mm2: Y[n0:n0+BT] = (hT.T @ w2) * gate ----
                for dt in range(DT):
                    pss = [ps2.tile([P, DTS], F32, name="ps2")
                           for _ in range(NCH)]
                    for hc in range(HC):
                        w2_sb = w2_pool.tile([P, DTS], MM_DT, name="w2sb")
                        nc.sync.dma_start(out=w2_sb[:], in_=w2t[dt, hc, :, :])
                        for ncq in range(NCH):
                            nc.tensor.matmul(
                                pss[ncq][:],
                                lhsT=ht_tiles[hc][:, ncq * P:(ncq + 1) * P],
                                rhs=w2_sb[:],
                                start=(hc == 0),
                                stop=(hc == HC - 1),
                            )
                    for ncq in range(NCH):
                        yo = yo_pool.tile([P, DTS], F32, name="yo")
                        gcol = (n0 + ncq * P) // P
                        nc.scalar.activation(
                            yo[:], pss[ncq][:],
                            mybir.ActivationFunctionType.Copy,
                            scale=g_sb[:, gcol:gcol + 1],
                        )
                        nc.sync.dma_start(
                            out=yg[n0 + ncq * P:n0 + (ncq + 1) * P,
                                   dt * DTS:(dt + 1) * DTS],
                            in_=yo[:],
                        )
    nc.compile()
    return nc


_NC_CACHE = None
TRACE = False
LAST_RESULTS = None


def _get_nc():
    global _NC_CACHE
    if _NC_CACHE is None:
        _NC_CACHE = build_nc()
    return _NC_CACHE


def kernel(x, gate_w, w1, b1, w2, b2):
    x = np.asarray(x, dtype=np.float32)
    gate_w = np.asarray(gate_w, dtype=np.float32)
    w1 = np.asarray(w1, dtype=np.float32)
    b1 = np.asarray(b1, dtype=np.float32)
    w2 = np.asarray(w2, dtype=np.float32)
    b2 = np.asarray(b2, dtype=np.float32)

    B, T, D = x.shape
    N = B * T
    xf = x.reshape(N, D)

    # ---- router (host; 0.05% of model FLOPs — this is the sharding step) ----
    logits = xf @ gate_w.T                           # [N, E]
    order = np.argsort(-logits, axis=1, kind="stable")
    i1, i2 = order[:, 0], order[:, 1]
    l1 = logits[np.arange(N), i1].astype(np.float64)
    l2 = logits[np.arange(N), i2].astype(np.float64)
    g1 = (1.0 / (1.0 + np.exp(l2 - l1))).astype(np.float32)
    g2 = (1.0 - g1).astype(np.float32)

    # ---- dispatch: gather per-expert tokens, pre-tile all inputs ----
    in_maps = []
    idx_per_e = []
    for e in range(E):
        sel1 = np.nonzero(i1 == e)[0]
        sel2 = np.nonzero(i2 == e)[0]
        idx = np.concatenate([sel1, sel2])
        gv = np.concatenate([g1[sel1], g2[sel2]])
        cnt = idx.shape[0]
        assert cnt <= C, f"expert {e} over capacity: {cnt} > {C}"
        idx_per_e.append(idx)

        xg = np.zeros((C, D), np.float32)
        xg[:cnt] = xf[idx]
        xgt = np.ascontiguousarray(
            xg.T.reshape(DC, P, C))               # [dc, p, n]
        w1t = np.ascontiguousarray(
            w1[e].reshape(DC, P, NHG, HG * P).transpose(2, 0, 1, 3))
        w2t = np.ascontiguousarray(
            w2[e].reshape(HC, P, DT, DTS).transpose(2, 0, 1, 3))
        b1t = np.ascontiguousarray(b1[e].reshape(HC, P).T)
        gfull = np.zeros(C, np.float32)
        gfull[:cnt] = gv
        gt = np.ascontiguousarray(gfull.reshape(C // P, P).T)
        in_maps.append(
            {"xgt": xgt, "w1t": w1t, "w2t": w2t, "b1t": b1t, "gt": gt})

    nc = _get_nc()
    res = bass_utils.run_bass_kernel_spmd(
        nc, in_maps, core_ids=list(range(N_CORES)), trace=TRACE)
    global LAST_RESULTS
    LAST_RESULTS = res

    # ---- combine (host): each token occurs in exactly 2 experts, never twice
    # in one, so fancy-index += is safe per expert ----
    out = np.zeros((N, D), np.float32)
    for e in range(E):
        idx = idx_per_e[e]
        out[idx] += res.results[e]["yg"][:idx.shape[0]]

    if np.any(b2):
        gate_full = np.zeros((N, E), np.float32)
        gate_full[np.arange(N), i1] = g1
        gate_full[np.arange(N), i2] = g2
        out += gate_full @ b2.reshape(E, D)

    return out.reshape(B, T, D)
